# revision 19
# baseline (speedup 1.0000x reference)
"""MHA SPMD kernel v6 for TRN2 (8 cores, head-parallel, mask-compacted keys).

v6 over v5:
- host-side key compaction: only unmasked keys (padded to SKP, a multiple
  of 128) go through K/V projection, scores, exp and AV.  Padding keys
  have x=0 -> k=0 -> score 0 -> exp 1, and are excluded from both the
  numerator and the softmax denominator by a 0/1 valid flag in the V
  "ones" column.  mask~Bernoulli(0.5) makes this a ~2x cut in attention
  work.
- scores for the two heads are issued back-to-back as PE row-tiles
  (rows 0-63 / 64-127) so they execute concurrently; one [128,2,IB]
  PSUM tile holds both heads' scores for a key chunk.
- exp runs as one ACT instruction per key chunk covering both heads
  (1024 free elements) to amortize the ~352-cycle ACT startup.
- AV for both heads interleaved per key chunk; per-head M=65 (64 V dims
  + valid column) gives the softmax denominator for free.

Output row mapping (per core c):
  y[b*SPC + il*64 + r, :] = out[b, il*IB + c*64 + r, :].
"""

from dataclasses import dataclass

import numpy as np

import concourse.bass as bass
import concourse.bacc as bacc
import concourse.mybir as mybir
import concourse.tile as tile
from concourse.masks import make_identity

F16 = mybir.dt.float16
F32 = mybir.dt.float32
NP_F16 = np.float16


@dataclass
class Cfg:
    B: int = 4
    S: int = 2048
    H: int = 1024
    nh: int = 16
    ncores: int = 8
    IB: int = 512
    SKP: int = 1152  # padded kept-key count (multiple of 128)

    @property
    def dk(self):
        return self.H // self.nh

    @property
    def R(self):
        return self.B * self.S

    @property
    def SPC(self):
        return self.S // self.ncores

    @property
    def KC(self):
        return self.H // 128

    @property
    def JCK(self):
        return self.SKP // 128

    @property
    def NIL(self):
        return self.S // self.IB


def build_nc(cfg: Cfg, loop_n: int = 0, fake_a2a: bool = False,
             phases=('proj', 'attn', 'a2a', 'out'),
             attn_parts=('sc', 'exp', 'av', 'norm')) -> bass.Bass:
    assert cfg.dk == 64
    B, S, H, R, IB = cfg.B, cfg.S, cfg.H, cfg.R, cfg.IB
    KC, JCK, SPC, SKP = cfg.KC, cfg.JCK, cfg.SPC, cfg.SKP
    NC = cfg.ncores
    NIL = cfg.NIL
    assert S % IB == 0 and SPC % 128 == 0 and SKP % 128 == 0

    nc = bacc.Bacc("TRN2")

    xt = nc.declare_dram_parameter("xt", [H, R], F16, isOutput=False)
    xkt = nc.declare_dram_parameter("xkt", [H, B * SKP], F16, isOutput=False)
    wq = nc.declare_dram_parameter("wq_t", [H, 128], F16, isOutput=False)
    wk = nc.declare_dram_parameter("wk_t", [H, 128], F16, isOutput=False)
    wv = nc.declare_dram_parameter("wv_t", [H, 128], F16, isOutput=False)
    wo = nc.declare_dram_parameter("wo_t", [H, H], F16, isOutput=False)
    vld = nc.declare_dram_parameter("valid16", [128, B * JCK], F16,
                                    isOutput=False)
    y = nc.declare_dram_parameter("y", [B * SPC, H], F16, isOutput=True)

    CW = IB // NC                      # columns per A2A chunk shard (64)
    cc_in = [
        [nc.dram_tensor(f"cc_in{b}_{il}", [NC * 128, CW], F16) for il in range(NIL)]
        for b in range(B)
    ]
    cc_out = [
        [nc.dram_tensor(f"cc_out{b}_{il}", [NC * 128, CW], F16) for il in range(NIL)]
        for b in range(B)
    ]

    xt_r = xt[:].rearrange("(kc p) i -> p kc i", p=128)
    xkt_r = xkt[:].rearrange("(kc p) i -> p kc i", p=128)
    wq_r = wq[:].rearrange("(kc p) m -> p kc m", p=128)
    wk_r = wk[:].rearrange("(kc p) m -> p kc m", p=128)
    wv_r = wv[:].rearrange("(kc p) m -> p kc m", p=128)
    wo_r = wo[:].rearrange("(kc p) n -> p kc n", p=128)

    with tile.TileContext(nc) as tc:
        with tc.tile_pool(name="persist", bufs=1) as persist:
            wq_sb = persist.tile([128, KC, 128], F16)
            wk_sb = persist.tile([128, KC, 128], F16)
            wv_sb = persist.tile([128, KC, 128], F16)
            wo_sb = persist.tile([128, KC, H], F16)
            nc.scalar.dma_start(out=wq_sb[:], in_=wq_r)
            nc.scalar.dma_start(out=wk_sb[:], in_=wk_r)
            nc.scalar.dma_start(out=wv_sb[:], in_=wv_r)

            qt_sb = [persist.tile([128, S], F16, name=f"qt{b}") for b in range(B)]
            kt_sb = [persist.tile([128, SKP], F16, name=f"kt{b}") for b in range(B)]
            v_sb = [
                persist.tile([128, JCK, 130], F16, name=f"v{b}") for b in range(B)
            ]
            ones65 = persist.tile([65, 64], F16)
            nc.vector.memset(ones65[64:65, :], 1.0)
            ident = persist.tile([128, 128], F16)
            make_identity(nc, ident)
            # valid flags -> the two per-head "ones" columns of V (once;
            # the steady-state loop never rewrites these columns)
            for b in range(B):
                vsl = vld[:, bass.ds(b * JCK, JCK)].rearrange(
                    "p (n o) -> p n o", o=1
                )
                nc.gpsimd.dma_start(out=v_sb[b][:, :, 64:65], in_=vsl)
                nc.gpsimd.dma_start(out=v_sb[b][:, :, 129:130], in_=vsl)

            with (
                tc.tile_pool(name="xtp", bufs=3) as xtp,
                tc.tile_pool(name="xkp", bufs=1) as xkp,
                tc.tile_pool(name="ep", bufs=2) as ep,
                tc.tile_pool(name="rp", bufs=4) as rp,
                tc.tile_pool(name="asb", bufs=4) as asb,
                tc.tile_pool(name="agp", bufs=2) as agp,
                tc.tile_pool(name="ysb", bufs=2) as ysb,
                tc.tile_pool(name="pmm", bufs=1, space="PSUM") as pmm,
                tc.tile_pool(name="ps", bufs=2, space="PSUM") as ps,
                tc.tile_pool(name="po", bufs=3, space="PSUM") as po,
            ):

                def q_units(b):
                    units = []
                    for ibl in range(S // IB):
                        isl = bass.ts(ibl, IB)
                        gsl = bass.ds(b * S + ibl * IB, IB)

                        def q_unit(b=b, isl=isl, gsl=gsl):
                            xt_t = xtp.tile(
                                [128, KC, IB], F16, tag="xt", name="xt_t"
                            )
                            nc.sync.dma_start(out=xt_t[:], in_=xt_r[:, :, gsl])
                            qp = pmm.tile([128, IB], F32, tag="mm", name="qp")
                            for kc in range(KC):
                                nc.tensor.matmul(
                                    qp[:], wq_sb[:, kc], xt_t[:, kc],
                                    start=(kc == 0), stop=(kc == KC - 1),
                                )
                            nc.vector.tensor_scalar_mul(
                                qt_sb[b][:, isl], qp[:], 0.125
                            )

                        units.append(q_unit)
                    return units

                def kv_units(b):
                    """K+V projection over the compacted (kept) keys.

                    One whole-batch xkt DMA (scalar HWDGE ring), then
                    per-512-block K and V matmul units.
                    """
                    units = []
                    xk_holder = {}

                    def kv_dma(b=b, xh=xk_holder):
                        xk_t = xkp.tile(
                            [128, KC, SKP], F16, tag="xk", name="xk_t"
                        )
                        gsl = bass.ds(b * SKP, SKP)
                        nc.scalar.dma_start(out=xk_t[:], in_=xkt_r[:, :, gsl])
                        xh["t"] = xk_t

                    units.append(kv_dma)
                    off = 0
                    while off < SKP:
                        n = min(IB, SKP - off)

                        def k_unit(b=b, off=off, n=n, xh=xk_holder):
                            kp = pmm.tile([128, IB], F32, tag="mm", name="kp")
                            for kc in range(KC):
                                nc.tensor.matmul(
                                    kp[:, 0:n], wk_sb[:, kc],
                                    xh["t"][:, kc, bass.ds(off, n)],
                                    start=(kc == 0), stop=(kc == KC - 1),
                                )
                            nc.vector.tensor_copy(
                                kt_sb[b][:, bass.ds(off, n)], kp[:, 0:n]
                            )

                        def v_unit(b=b, off=off, n=n, xh=xk_holder):
                            vtp = pmm.tile([128, IB], F32, tag="mm", name="vtp")
                            for kc in range(KC):
                                nc.tensor.matmul(
                                    vtp[:, 0:n], wv_sb[:, kc],
                                    xh["t"][:, kc, bass.ds(off, n)],
                                    start=(kc == 0), stop=(kc == KC - 1),
                                )
                            vt16 = xtp.tile(
                                [128, IB], F16, tag="vt16", name="vt16"
                            )
                            for t in range(n // 128):
                                nc.vector.tensor_copy(
                                    vt16[:, bass.ts(t, 128)],
                                    vtp[:, bass.ts(t, 128)],
                                )
                                vp = pmm.tile(
                                    [128, 128], F16, tag="mm", name="vp"
                                )
                                nc.tensor.transpose(
                                    vp[:], vt16[:, bass.ts(t, 128)], ident[:]
                                )
                                ch = off // 128 + t
                                nc.vector.tensor_copy(
                                    v_sb[b][:, ch, 0:64], vp[:, 0:64]
                                )
                                nc.vector.tensor_copy(
                                    v_sb[b][:, ch, 65:129], vp[:, 64:128]
                                )

                        units += [k_unit, v_unit]
                        off += n
                    return units

                def out_proj_units(b):
                    units = []
                    for it in range(SPC // 128):
                        holder = {}

                        def u0(b=b, it=it, hd=holder):
                            ag_t = agp.tile(
                                [128, KC, 128], F16, tag="ag", name="ag_t"
                            )
                            for half in range(128 // CW):
                                il = it * (128 // CW) + half
                                cc_r = cc_out[b][il][:].rearrange(
                                    "(kc p) i -> p kc i", p=128
                                )
                                nc.sync.dma_start(
                                    out=ag_t[:, :, bass.ts(half, CW)], in_=cc_r
                                )
                            y_t = ysb.tile([128, H], F16, tag="y", name="y_t")
                            hd["ag"], hd["y"] = ag_t, y_t
                            yp = pmm.tile([128, 512], F32, tag="mm", name="yp")
                            for kc in range(KC):
                                nc.tensor.matmul(
                                    yp[:], ag_t[:, kc], wo_sb[:, kc, 0:512],
                                    start=(kc == 0), stop=(kc == KC - 1),
                                )
                            nc.vector.tensor_copy(y_t[:, 0:512], yp[:])

                        def u1(b=b, it=it, hd=holder):
                            yp = pmm.tile([128, 512], F32, tag="mm", name="yp")
                            for kc in range(KC):
                                nc.tensor.matmul(
                                    yp[:], hd["ag"][:, kc],
                                    wo_sb[:, kc, 512:1024],
                                    start=(kc == 0), stop=(kc == KC - 1),
                                )
                            nc.vector.tensor_copy(hd["y"][:, 512:1024], yp[:])
                            nc.scalar.dma_start(
                                out=y[bass.ds(b * SPC + it * 128, 128), :],
                                in_=hd["y"][:],
                            )

                        units += [u0, u1]
                    return units

                def attn_batch(b, fillers, pre_av=0):
                    fi = 0

                    def fill(n=1):
                        nonlocal fi
                        n = min(n, len(fillers) - fi)
                        for _ in range(n):
                            fillers[fi]()
                            fi += 1

                    nblk = NIL * 2
                    per_blk = -(-len(fillers) // nblk) if fillers else 0
                    for il in range(NIL):
                        qsl = bass.ts(il, IB)
                        e_t = ep.tile([128, JCK, 2, IB], F16, tag="e", name="e_t")
                        # scores + exp, both heads row-tiled per key chunk
                        for jc in range(JCK):
                            if "sc" in attn_parts:
                                sp = ps.tile(
                                    [128, 2, IB], F32, tag="sp", name="sp"
                                )
                                for h in range(2):
                                    hsl = bass.ds(h * 64, 64)
                                    nc.tensor.matmul(
                                        sp[:, h],
                                        kt_sb[b][hsl, bass.ts(jc, 128)],
                                        qt_sb[b][hsl, qsl],
                                        start=True, stop=True,
                                    )
                                if "exp" in attn_parts:
                                    nc.scalar.activation(
                                        e_t[:, jc], sp[:],
                                        mybir.ActivationFunctionType.Exp,
                                    )
                            if jc == JCK // 2:
                                fill((per_blk + 1) // 2)
                        if il == 0 and fi < pre_av:
                            # data-dependency: units the first AV loop needs
                            fill(pre_av - fi)
                        # AV per head (heads sequential so block il's norm
                        # chain hides under block il+1's first AV pass)
                        if "av" in attn_parts:
                            o2, r16s = [], []
                            for h in range(2):
                                o2.append(po.tile(
                                    [65, IB], F32, tag="oav", name=f"o2_{h}"
                                ))
                                for jc in range(JCK):
                                    nc.tensor.matmul(
                                        o2[h][:],
                                        v_sb[b][:, jc, bass.ds(h * 65, 65)],
                                        e_t[:, jc, h],
                                        start=(jc == 0), stop=(jc == JCK - 1),
                                    )
                                    if jc == JCK // 2:
                                        fill(per_blk // 2)
                                if "norm" in attn_parts:
                                    # early: reciprocal runs while the other
                                    # head's AV pass occupies the PE
                                    r32 = rp.tile(
                                        [65, IB], F32, tag="r32", name="r32"
                                    )
                                    nc.vector.reciprocal(
                                        r32[64:65, :], o2[h][64:65, :]
                                    )
                                    r16 = rp.tile(
                                        [65, IB], F16, tag="r16", name="r16"
                                    )
                                    nc.vector.tensor_copy(
                                        r16[64:65, :], r32[64:65, :]
                                    )
                                    r16s.append(r16)
                        if "norm" in attn_parts:
                            a_t = []
                            for h in range(2):
                                rb = ps.tile([64, IB], F32, tag="sp", name="rb")
                                nc.tensor.matmul(
                                    rb[:], ones65[64:65, :], r16s[h][64:65, :],
                                    start=True, stop=True,
                                )
                                rb_sb = rp.tile([64, IB], F32, tag="rbs", name="rb_sb")
                                nc.vector.tensor_copy(rb_sb[:], rb[:])
                                at = asb.tile([64, IB], F16, tag="a", name=f"a_{h}")
                                nc.vector.tensor_mul(
                                    at[:], o2[h][0:64, :], rb_sb[:]
                                )
                                a_t.append(at)
                        if has("a2a"):
                            a2a_chunk(b, il, a_t)
                    fill(len(fillers))

                def a2a_chunk(b, il, a_t):
                    # cc_in[b][il][j*128 + h*64 + p, i] = a_t[h][p, j*CW + i]
                    for h in range(2):
                        dst = cc_in[b][il][:].rearrange(
                            "(j two p) i -> two p j i", j=NC, two=2
                        )[h]
                        src = a_t[h][:].rearrange("p (j i) -> p j i", j=NC)
                        nc.scalar.dma_start(out=dst, in_=src)
                    if fake_a2a:
                        nc.gpsimd.dma_start(
                            out=cc_out[b][il][:], in_=cc_in[b][il][:]
                        )
                    else:
                        nc.gpsimd.collective_compute(
                            "AllToAll",
                            mybir.AluOpType.bypass,
                            replica_groups=[list(range(NC))],
                            ins=[cc_in[b][il][:]],
                            outs=[cc_out[b][il][:]],
                        )

                has = lambda p: p in phases

                def whole_kernel():
                    if not has("attn"):
                        if has("proj"):
                            for b in range(B):
                                for u in q_units(b) + kv_units(b):
                                    u()
                        if has("out"):
                            nc.scalar.dma_start(out=wo_sb[:], in_=wo_r)
                            for b in range(B):
                                for u in out_proj_units(b):
                                    u()
                        return
                    kv0 = kv_units(0) if has("proj") else []
                    if has("proj"):
                        # prologue: batch 0's Q, its kv DMA and K units
                        # (V units fill during attn 0, before its first AV)
                        for u in q_units(0) + kv0[0:1] + kv0[1::2]:
                            u()
                    nc.scalar.dma_start(out=wo_sb[:], in_=wo_r)
                    for b in range(B):
                        fillers = []
                        pre_av = 0
                        if has("proj"):
                            if b == 0:
                                fillers += kv0[2::2]
                                pre_av = len(fillers)
                            if b + 1 < B:
                                fillers += q_units(b + 1) + kv_units(b + 1)
                        if has("out") and b == B - 1:
                            for pb in range(B - 1):
                                fillers += out_proj_units(pb)
                        attn_batch(b, fillers, pre_av=pre_av)
                    if has("out"):
                        for u in out_proj_units(B - 1):
                            u()

                def attn_prereq():
                    for b in range(B):
                        for u in q_units(b) + kv_units(b):
                            u()

                if loop_n > 0:
                    if has("attn") and not has("proj"):
                        attn_prereq()
                    with tc.For_i(0, loop_n):
                        whole_kernel()
                else:
                    whole_kernel()

    nc.finalize()
    return nc


# ---------------------------------------------------------------------------


def compute_skp(mask, B, S):
    m = (np.asarray(mask).reshape(B, S) != 0)
    kept = int(m.sum(1).max())
    skp = max(128, -(-kept // 128) * 128)
    return min(skp, S)


def make_inputs(cfg: Cfg, x, mask, Wq, Wk, Wv, Wo):
    B, S, H, NC, SKP, JCK = cfg.B, cfg.S, cfg.H, cfg.ncores, cfg.SKP, cfg.JCK
    x = np.asarray(x, dtype=np.float32)
    xt = np.ascontiguousarray(x.reshape(B * S, H).T.astype(NP_F16))
    wo_t = np.ascontiguousarray(np.asarray(Wo).T.astype(NP_F16))

    m = (np.asarray(mask).reshape(B, S) != 0)
    xk = np.zeros((B, SKP, H), np.float32)
    valid = np.zeros((B, JCK * 128), NP_F16)
    for b in range(B):
        idx = np.nonzero(m[b])[0]
        assert len(idx) <= SKP, (len(idx), SKP)
        xk[b, : len(idx)] = x[b, idx]
        valid[b, : len(idx)] = 1.0
    xkt = np.ascontiguousarray(xk.reshape(B * SKP, H).T.astype(NP_F16))
    # valid16[p, b*JCK + c] = valid[b, c*128 + p]
    v16 = np.ascontiguousarray(
        valid.reshape(B * JCK, 128).T.astype(NP_F16)
    )

    ins = []
    for c in range(NC):
        blk = slice(c * 128, (c + 1) * 128)
        ins.append(
            {
                "xt": xt,
                "xkt": xkt,
                "wq_t": np.ascontiguousarray(np.asarray(Wq)[blk, :].T.astype(NP_F16)),
                "wk_t": np.ascontiguousarray(np.asarray(Wk)[blk, :].T.astype(NP_F16)),
                "wv_t": np.ascontiguousarray(np.asarray(Wv)[blk, :].T.astype(NP_F16)),
                "wo_t": wo_t,
                "valid16": v16,
            }
        )
    return ins


def assemble_output(cfg: Cfg, per_core_y, bo):
    B, S, H, SPC, IB = cfg.B, cfg.S, cfg.H, cfg.SPC, cfg.IB
    NC = cfg.ncores
    CW = IB // NC
    NIL = S // IB
    out = np.empty((B, S, H), np.float32)
    for c, yc in enumerate(per_core_y):
        yc = np.asarray(yc).reshape(B, NIL, CW, H)
        for b in range(B):
            for il in range(NIL):
                out[b, il * IB + c * CW : il * IB + (c + 1) * CW] = yc[b, il]
    out += np.asarray(bo, dtype=np.float32)[None, None, :]
    return out


def reference_np(cfg: Cfg, x, mask, Wq, Wk, Wv, Wo, bo):
    B, S, H, nh, dk = cfg.B, cfg.S, cfg.H, cfg.nh, cfg.dk
    xf = np.asarray(x).reshape(B * S, H).astype(np.float64)
    out = np.zeros((B, S, H), np.float64)
    for b in range(B):
        xb = xf[b * S : (b + 1) * S]
        mrow = np.asarray(mask).reshape(B, S)[b]
        A = np.zeros((S, H), np.float64)
        for h in range(nh):
            q = xb @ np.asarray(Wq)[h * dk : (h + 1) * dk].T.astype(np.float64) / np.sqrt(dk)
            k = xb @ np.asarray(Wk)[h * dk : (h + 1) * dk].T.astype(np.float64)
            v = xb @ np.asarray(Wv)[h * dk : (h + 1) * dk].T.astype(np.float64)
            sc = q @ k.T
            sc = np.where(mrow[None, :] == 0, -1e9, sc)
            e = np.exp(sc - sc.max(-1, keepdims=True))
            p = e / e.sum(-1, keepdims=True)
            A[:, h * dk : (h + 1) * dk] = p @ v
        out[b] = A @ np.asarray(Wo).T.astype(np.float64)
    return (out + np.asarray(bo)[None, None, :]).astype(np.float32)


# ---------------------------------------------------------------------------
# harness entry point: full inputs in, full output out

_CACHED = {}


def kernel(x, mask, Wq, Wk, Wv, Wo, bo):
    """Multi-head attention on 8 TRN2 NeuronCores (head-parallel TP).

    Sharding: 2 heads per core (Wq/Wk/Wv split by head rows); keys are
    compacted host-side using the (host-visible) key mask so only kept
    keys flow through K/V projection, scores, exp and AV; scores/softmax
    /AV run in keys-on-partition layout with a valid-flag column in the
    V-augmented matmul providing the softmax denominator; 16 small
    AllToAll collectives redistribute the head-sharded attention output
    to seq-sharded form as each 512-query block completes; each core
    then computes its 1/8 of output rows against full Wo.
    """
    from concourse.bass_utils import run_bass_kernel_spmd

    x = np.ascontiguousarray(np.asarray(x, dtype=np.float32))
    mask = np.asarray(mask)
    Wq = np.asarray(Wq, dtype=np.float32)
    Wk = np.asarray(Wk, dtype=np.float32)
    Wv = np.asarray(Wv, dtype=np.float32)
    Wo = np.asarray(Wo, dtype=np.float32)
    bo = np.asarray(bo, dtype=np.float32)

    B, S, H = x.shape
    skp = compute_skp(mask, B, S)
    cfg = Cfg(B=B, S=S, H=H, SKP=skp)
    if _CACHED.get("skp") != skp:
        _CACHED["nc"] = build_nc(cfg)
        _CACHED["skp"] = skp
    nc = _CACHED["nc"]

    ins = make_inputs(cfg, x, mask, Wq, Wk, Wv, Wo)
    res = run_bass_kernel_spmd(nc, ins, list(range(cfg.ncores)))
    ys = [res.results[c]["y"] for c in range(cfg.ncores)]
    return assemble_output(cfg, ys, bo).astype(np.float32)


# revision 22
# speedup vs baseline: 1.0622x; 1.0622x over previous
"""MHA SPMD kernel v6 for TRN2 (8 cores, head-parallel, mask-compacted keys).

v6 over v5:
- host-side key compaction: only unmasked keys (padded to SKP, a multiple
  of 128) go through K/V projection, scores, exp and AV.  Padding keys
  have x=0 -> k=0 -> score 0 -> exp 1, and are excluded from both the
  numerator and the softmax denominator by a 0/1 valid flag in the V
  "ones" column.  mask~Bernoulli(0.5) makes this a ~2x cut in attention
  work.
- scores for the two heads are issued back-to-back as PE row-tiles
  (rows 0-63 / 64-127) so they execute concurrently; one [128,2,IB]
  PSUM tile holds both heads' scores for a key chunk.
- exp runs as one ACT instruction per key chunk covering both heads
  (1024 free elements) to amortize the ~352-cycle ACT startup.
- AV for both heads interleaved per key chunk; per-head M=65 (64 V dims
  + valid column) gives the softmax denominator for free.

Output row mapping (per core c):
  y[b*SPC + il*64 + r, :] = out[b, il*IB + c*64 + r, :].
"""

from dataclasses import dataclass

import numpy as np

import concourse.bass as bass
import concourse.bacc as bacc
import concourse.mybir as mybir
import concourse.tile as tile
from concourse.masks import make_identity

F16 = mybir.dt.float16
F32 = mybir.dt.float32
NP_F16 = np.float16


@dataclass
class Cfg:
    B: int = 4
    S: int = 2048
    H: int = 1024
    nh: int = 16
    ncores: int = 8
    IB: int = 512
    SKP: int = 1152  # padded kept-key count (multiple of 128)

    @property
    def dk(self):
        return self.H // self.nh

    @property
    def R(self):
        return self.B * self.S

    @property
    def SPC(self):
        return self.S // self.ncores

    @property
    def KC(self):
        return self.H // 128

    @property
    def JCK(self):
        return self.SKP // 128

    @property
    def NIL(self):
        return self.S // self.IB


def build_nc(cfg: Cfg, loop_n: int = 0, fake_a2a: bool = False,
             phases=('proj', 'attn', 'a2a', 'out'),
             attn_parts=('sc', 'exp', 'av', 'norm')) -> bass.Bass:
    assert cfg.dk == 64
    B, S, H, R, IB = cfg.B, cfg.S, cfg.H, cfg.R, cfg.IB
    KC, JCK, SPC, SKP = cfg.KC, cfg.JCK, cfg.SPC, cfg.SKP
    NC = cfg.ncores
    NIL = cfg.NIL
    assert S % IB == 0 and SPC % 128 == 0 and SKP % 128 == 0

    nc = bacc.Bacc("TRN2")

    xt = nc.declare_dram_parameter("xt", [H, R], F16, isOutput=False)
    xkt = nc.declare_dram_parameter("xkt", [H, B * SKP], F16, isOutput=False)
    wq = nc.declare_dram_parameter("wq_t", [H, 128], F16, isOutput=False)
    wk = nc.declare_dram_parameter("wk_t", [H, 128], F16, isOutput=False)
    wv = nc.declare_dram_parameter("wv_t", [H, 128], F16, isOutput=False)
    wo = nc.declare_dram_parameter("wo_t", [H, H], F16, isOutput=False)
    vld = nc.declare_dram_parameter("valid16", [128, B * JCK], F16,
                                    isOutput=False)
    y = nc.declare_dram_parameter("y", [B * SPC, H], F16, isOutput=True)

    CW = IB // NC                      # columns per A2A chunk shard (64)
    cc_in = [
        [nc.dram_tensor(f"cc_in{b}_{il}", [NC * 128, CW], F16) for il in range(NIL)]
        for b in range(B)
    ]
    cc_out = [
        [nc.dram_tensor(f"cc_out{b}_{il}", [NC * 128, CW], F16) for il in range(NIL)]
        for b in range(B)
    ]

    xt_r = xt[:].rearrange("(kc p) i -> p kc i", p=128)
    xkt_r = xkt[:].rearrange("(kc p) i -> p kc i", p=128)
    wq_r = wq[:].rearrange("(kc p) m -> p kc m", p=128)
    wk_r = wk[:].rearrange("(kc p) m -> p kc m", p=128)
    wv_r = wv[:].rearrange("(kc p) m -> p kc m", p=128)
    wo_r = wo[:].rearrange("(kc p) n -> p kc n", p=128)

    with tile.TileContext(nc) as tc:
        with tc.tile_pool(name="persist", bufs=1) as persist:
            wq_sb = persist.tile([128, KC, 128], F16)
            wk_sb = persist.tile([128, KC, 128], F16)
            wv_sb = persist.tile([128, KC, 128], F16)
            wo_sb = persist.tile([128, KC, H], F16)
            nc.scalar.dma_start(out=wq_sb[:], in_=wq_r)
            nc.scalar.dma_start(out=wk_sb[:], in_=wk_r)
            nc.scalar.dma_start(out=wv_sb[:], in_=wv_r)

            qt_sb = [persist.tile([128, S], F16, name=f"qt{b}") for b in range(B)]
            kt_sb = [persist.tile([128, SKP], F16, name=f"kt{b}") for b in range(B)]
            v_sb = [
                persist.tile([128, JCK, 130], F16, name=f"v{b}") for b in range(B)
            ]
            ones65 = persist.tile([65, 64], F16)
            nc.vector.memset(ones65[64:65, :], 1.0)
            ident = persist.tile([128, 128], F16)
            make_identity(nc, ident)
            # valid flags -> the two per-head "ones" columns of V (once;
            # the steady-state loop never rewrites these columns)
            for b in range(B):
                vsl = vld[:, bass.ds(b * JCK, JCK)].rearrange(
                    "p (n o) -> p n o", o=1
                )
                nc.gpsimd.dma_start(out=v_sb[b][:, :, 64:65], in_=vsl)
                nc.gpsimd.dma_start(out=v_sb[b][:, :, 129:130], in_=vsl)

            with (
                tc.tile_pool(name="xtp", bufs=3) as xtp,
                tc.tile_pool(name="xkp", bufs=1) as xkp,
                tc.tile_pool(name="ep", bufs=2) as ep,
                tc.tile_pool(name="rp", bufs=4) as rp,
                tc.tile_pool(name="asb", bufs=4) as asb,
                tc.tile_pool(name="agp", bufs=2) as agp,
                tc.tile_pool(name="ysb", bufs=2) as ysb,
                tc.tile_pool(name="pmm", bufs=1, space="PSUM") as pmm,
                tc.tile_pool(name="ps", bufs=2, space="PSUM") as ps,
                tc.tile_pool(name="po", bufs=2, space="PSUM") as po,
                tc.tile_pool(name="prb", bufs=1, space="PSUM") as prb,
            ):

                def q_units(b):
                    units = []
                    for ibl in range(S // IB):
                        isl = bass.ts(ibl, IB)
                        gsl = bass.ds(b * S + ibl * IB, IB)

                        def q_unit(b=b, isl=isl, gsl=gsl):
                            xt_t = xtp.tile(
                                [128, KC, IB], F16, tag="xt", name="xt_t"
                            )
                            nc.sync.dma_start(out=xt_t[:], in_=xt_r[:, :, gsl])
                            qp = pmm.tile([128, IB], F32, tag="mm", name="qp")
                            for kc in range(KC):
                                nc.tensor.matmul(
                                    qp[:], wq_sb[:, kc], xt_t[:, kc],
                                    start=(kc == 0), stop=(kc == KC - 1),
                                )
                            nc.vector.tensor_scalar_mul(
                                qt_sb[b][:, isl], qp[:], 0.125
                            )

                        units.append(q_unit)
                    return units

                def kv_units(b):
                    """K+V projection over the compacted (kept) keys.

                    One whole-batch xkt DMA (scalar HWDGE ring), then
                    per-512-block K and V matmul units.
                    """
                    units = []
                    xk_holder = {}

                    def kv_dma(b=b, xh=xk_holder):
                        xk_t = xkp.tile(
                            [128, KC, SKP], F16, tag="xk", name="xk_t"
                        )
                        gsl = bass.ds(b * SKP, SKP)
                        nc.scalar.dma_start(out=xk_t[:], in_=xkt_r[:, :, gsl])
                        xh["t"] = xk_t

                    units.append(kv_dma)
                    off = 0
                    while off < SKP:
                        n = min(IB, SKP - off)

                        def k_unit(b=b, off=off, n=n, xh=xk_holder):
                            kp = pmm.tile([128, IB], F32, tag="mm", name="kp")
                            for kc in range(KC):
                                nc.tensor.matmul(
                                    kp[:, 0:n], wk_sb[:, kc],
                                    xh["t"][:, kc, bass.ds(off, n)],
                                    start=(kc == 0), stop=(kc == KC - 1),
                                )
                            nc.vector.tensor_copy(
                                kt_sb[b][:, bass.ds(off, n)], kp[:, 0:n]
                            )

                        def v_unit(b=b, off=off, n=n, xh=xk_holder):
                            vtp = pmm.tile([128, IB], F32, tag="mm", name="vtp")
                            for kc in range(KC):
                                nc.tensor.matmul(
                                    vtp[:, 0:n], wv_sb[:, kc],
                                    xh["t"][:, kc, bass.ds(off, n)],
                                    start=(kc == 0), stop=(kc == KC - 1),
                                )
                            vt16 = xtp.tile(
                                [128, IB], F16, tag="vt16", name="vt16"
                            )
                            for t in range(n // 128):
                                nc.vector.tensor_copy(
                                    vt16[:, bass.ts(t, 128)],
                                    vtp[:, bass.ts(t, 128)],
                                )
                                vp = pmm.tile(
                                    [128, 128], F16, tag="mm", name="vp"
                                )
                                nc.tensor.transpose(
                                    vp[:], vt16[:, bass.ts(t, 128)], ident[:]
                                )
                                ch = off // 128 + t
                                nc.vector.tensor_copy(
                                    v_sb[b][:, ch, 0:64], vp[:, 0:64]
                                )
                                nc.vector.tensor_copy(
                                    v_sb[b][:, ch, 65:129], vp[:, 64:128]
                                )

                        units += [k_unit, v_unit]
                        off += n
                    return units

                def out_proj_units(b):
                    units = []
                    for it in range(SPC // 128):
                        holder = {}

                        def u0(b=b, it=it, hd=holder):
                            ag_t = agp.tile(
                                [128, KC, 128], F16, tag="ag", name="ag_t"
                            )
                            for half in range(128 // CW):
                                il = it * (128 // CW) + half
                                cc_r = cc_out[b][il][:].rearrange(
                                    "(kc p) i -> p kc i", p=128
                                )
                                nc.sync.dma_start(
                                    out=ag_t[:, :, bass.ts(half, CW)], in_=cc_r
                                )
                            y_t = ysb.tile([128, H], F16, tag="y", name="y_t")
                            hd["ag"], hd["y"] = ag_t, y_t
                            yp = pmm.tile([128, 512], F32, tag="mm", name="yp")
                            for kc in range(KC):
                                nc.tensor.matmul(
                                    yp[:], ag_t[:, kc], wo_sb[:, kc, 0:512],
                                    start=(kc == 0), stop=(kc == KC - 1),
                                )
                            nc.vector.tensor_copy(y_t[:, 0:512], yp[:])

                        def u1(b=b, it=it, hd=holder):
                            yp = pmm.tile([128, 512], F32, tag="mm", name="yp")
                            for kc in range(KC):
                                nc.tensor.matmul(
                                    yp[:], hd["ag"][:, kc],
                                    wo_sb[:, kc, 512:1024],
                                    start=(kc == 0), stop=(kc == KC - 1),
                                )
                            nc.vector.tensor_copy(hd["y"][:, 512:1024], yp[:])
                            nc.scalar.dma_start(
                                out=y[bass.ds(b * SPC + it * 128, 128), :],
                                in_=hd["y"][:],
                            )

                        units += [u0, u1]
                    return units

                def attn_batch(b, fillers, pre_av=0):
                    fi = 0

                    def fill(n=1):
                        nonlocal fi
                        n = min(n, len(fillers) - fi)
                        for _ in range(n):
                            fillers[fi]()
                            fi += 1

                    def scores_chunk(il, jc, e_t):
                        qsl = bass.ts(il, IB)
                        sp = ps.tile([128, 2, IB], F32, tag="sp", name="sp")
                        for h in range(2):
                            hsl = bass.ds(h * 64, 64)
                            nc.tensor.matmul(
                                sp[:, h],
                                kt_sb[b][hsl, bass.ts(jc, 128)],
                                qt_sb[b][hsl, qsl],
                                start=True, stop=True,
                            )
                        if "exp" in attn_parts:
                            nc.scalar.activation(
                                e_t[:, jc], sp[:],
                                mybir.ActivationFunctionType.Exp,
                            )

                    # software pipeline: scores/exp for block il+1 are
                    # emitted interleaved with AV for block il, so ACT
                    # streams exps continuously while PE does AV work.
                    nblk = NIL + 1
                    per_blk = -(-len(fillers) // nblk) if fillers else 0
                    e_ts = {}
                    if "sc" in attn_parts:
                        e_ts[0] = ep.tile(
                            [128, JCK, 2, IB], F16, tag="e", name="e_t"
                        )
                        for jc in range(JCK):
                            scores_chunk(0, jc, e_ts[0])
                            if jc == JCK // 2:
                                fill(per_blk)
                    if fi < pre_av:
                        # data-dependency: units the first AV loop needs
                        fill(pre_av - fi)
                    for il in range(NIL):
                        e_t = e_ts.pop(il, None)
                        if "sc" in attn_parts and il + 1 < NIL:
                            e_ts[il + 1] = ep.tile(
                                [128, JCK, 2, IB], F16, tag="e", name="e_t"
                            )
                        if "av" in attn_parts:
                            o2 = [
                                po.tile([65, IB], F32, tag="oav", name=f"o2_{h}")
                                for h in range(2)
                            ]
                            for jc in range(JCK):
                                if "sc" in attn_parts and il + 1 < NIL:
                                    scores_chunk(il + 1, jc, e_ts[il + 1])
                                for h in range(2):
                                    nc.tensor.matmul(
                                        o2[h][:],
                                        v_sb[b][:, jc, bass.ds(h * 65, 65)],
                                        e_t[:, jc, h],
                                        start=(jc == 0), stop=(jc == JCK - 1),
                                    )
                                if jc == JCK // 2:
                                    fill(per_blk)
                        elif "sc" in attn_parts and il + 1 < NIL:
                            for jc in range(JCK):
                                scores_chunk(il + 1, jc, e_ts[il + 1])
                        if "norm" in attn_parts:
                            a_t = []
                            r16s = []
                            for h in range(2):
                                r32 = rp.tile([65, IB], F32, tag="r32", name="r32")
                                nc.vector.reciprocal(
                                    r32[64:65, :], o2[h][64:65, :]
                                )
                                r16 = rp.tile([65, IB], F16, tag="r16", name="r16")
                                nc.vector.tensor_copy(
                                    r16[64:65, :], r32[64:65, :]
                                )
                                r16s.append(r16)
                            # PE cover for the reciprocal chain latency so
                            # the rb matmuls don't stall the PE queue
                            fill(1)
                            for h in range(2):
                                rb = prb.tile([64, IB], F32, tag="rb", name="rb")
                                nc.tensor.matmul(
                                    rb[:], ones65[64:65, :], r16s[h][64:65, :],
                                    start=True, stop=True,
                                )
                                rb_sb = rp.tile([64, IB], F32, tag="rbs", name="rb_sb")
                                nc.vector.tensor_copy(rb_sb[:], rb[:])
                                at = asb.tile([64, IB], F16, tag="a", name=f"a_{h}")
                                nc.vector.tensor_mul(
                                    at[:], o2[h][0:64, :], rb_sb[:]
                                )
                                a_t.append(at)
                        if has("a2a"):
                            a2a_chunk(b, il, a_t)
                    fill(len(fillers))

                def a2a_chunk(b, il, a_t):
                    # cc_in[b][il][j*128 + h*64 + p, i] = a_t[h][p, j*CW + i]
                    for h in range(2):
                        dst = cc_in[b][il][:].rearrange(
                            "(j two p) i -> two p j i", j=NC, two=2
                        )[h]
                        src = a_t[h][:].rearrange("p (j i) -> p j i", j=NC)
                        nc.scalar.dma_start(out=dst, in_=src)
                    if fake_a2a:
                        nc.gpsimd.dma_start(
                            out=cc_out[b][il][:], in_=cc_in[b][il][:]
                        )
                    else:
                        nc.gpsimd.collective_compute(
                            "AllToAll",
                            mybir.AluOpType.bypass,
                            replica_groups=[list(range(NC))],
                            ins=[cc_in[b][il][:]],
                            outs=[cc_out[b][il][:]],
                        )

                has = lambda p: p in phases

                def whole_kernel():
                    if not has("attn"):
                        if has("proj"):
                            for b in range(B):
                                for u in q_units(b) + kv_units(b):
                                    u()
                        if has("out"):
                            nc.scalar.dma_start(out=wo_sb[:], in_=wo_r)
                            for b in range(B):
                                for u in out_proj_units(b):
                                    u()
                        return
                    kv0 = kv_units(0) if has("proj") else []
                    if has("proj"):
                        # prologue: batch 0's Q, its kv DMA and K units
                        # (V units fill during attn 0, before its first AV)
                        for u in q_units(0) + kv0[0:1] + kv0[1::2]:
                            u()
                    nc.scalar.dma_start(out=wo_sb[:], in_=wo_r)
                    for b in range(B):
                        fillers = []
                        pre_av = 0
                        if has("proj"):
                            if b == 0:
                                fillers += kv0[2::2]
                                pre_av = len(fillers)
                            if b + 1 < B:
                                fillers += q_units(b + 1) + kv_units(b + 1)
                        if has("out") and b == B - 1:
                            for pb in range(B - 1):
                                fillers += out_proj_units(pb)
                        attn_batch(b, fillers, pre_av=pre_av)
                    if has("out"):
                        for u in out_proj_units(B - 1):
                            u()

                def attn_prereq():
                    for b in range(B):
                        for u in q_units(b) + kv_units(b):
                            u()

                if loop_n > 0:
                    if has("attn") and not has("proj"):
                        attn_prereq()
                    with tc.For_i(0, loop_n):
                        whole_kernel()
                else:
                    whole_kernel()

    nc.finalize()
    return nc


# ---------------------------------------------------------------------------


def compute_skp(mask, B, S):
    m = (np.asarray(mask).reshape(B, S) != 0)
    kept = int(m.sum(1).max())
    skp = max(128, -(-kept // 128) * 128)
    return min(skp, S)


def make_inputs(cfg: Cfg, x, mask, Wq, Wk, Wv, Wo):
    B, S, H, NC, SKP, JCK = cfg.B, cfg.S, cfg.H, cfg.ncores, cfg.SKP, cfg.JCK
    x = np.asarray(x, dtype=np.float32)
    xt = np.ascontiguousarray(x.reshape(B * S, H).T.astype(NP_F16))
    wo_t = np.ascontiguousarray(np.asarray(Wo).T.astype(NP_F16))

    m = (np.asarray(mask).reshape(B, S) != 0)
    xk = np.zeros((B, SKP, H), np.float32)
    valid = np.zeros((B, JCK * 128), NP_F16)
    for b in range(B):
        idx = np.nonzero(m[b])[0]
        assert len(idx) <= SKP, (len(idx), SKP)
        xk[b, : len(idx)] = x[b, idx]
        valid[b, : len(idx)] = 1.0
    xkt = np.ascontiguousarray(xk.reshape(B * SKP, H).T.astype(NP_F16))
    # valid16[p, b*JCK + c] = valid[b, c*128 + p]
    v16 = np.ascontiguousarray(
        valid.reshape(B * JCK, 128).T.astype(NP_F16)
    )

    ins = []
    for c in range(NC):
        blk = slice(c * 128, (c + 1) * 128)
        ins.append(
            {
                "xt": xt,
                "xkt": xkt,
                "wq_t": np.ascontiguousarray(np.asarray(Wq)[blk, :].T.astype(NP_F16)),
                "wk_t": np.ascontiguousarray(np.asarray(Wk)[blk, :].T.astype(NP_F16)),
                "wv_t": np.ascontiguousarray(np.asarray(Wv)[blk, :].T.astype(NP_F16)),
                "wo_t": wo_t,
                "valid16": v16,
            }
        )
    return ins


def assemble_output(cfg: Cfg, per_core_y, bo):
    B, S, H, SPC, IB = cfg.B, cfg.S, cfg.H, cfg.SPC, cfg.IB
    NC = cfg.ncores
    CW = IB // NC
    NIL = S // IB
    out = np.empty((B, S, H), np.float32)
    for c, yc in enumerate(per_core_y):
        yc = np.asarray(yc).reshape(B, NIL, CW, H)
        for b in range(B):
            for il in range(NIL):
                out[b, il * IB + c * CW : il * IB + (c + 1) * CW] = yc[b, il]
    out += np.asarray(bo, dtype=np.float32)[None, None, :]
    return out


def reference_np(cfg: Cfg, x, mask, Wq, Wk, Wv, Wo, bo):
    B, S, H, nh, dk = cfg.B, cfg.S, cfg.H, cfg.nh, cfg.dk
    xf = np.asarray(x).reshape(B * S, H).astype(np.float64)
    out = np.zeros((B, S, H), np.float64)
    for b in range(B):
        xb = xf[b * S : (b + 1) * S]
        mrow = np.asarray(mask).reshape(B, S)[b]
        A = np.zeros((S, H), np.float64)
        for h in range(nh):
            q = xb @ np.asarray(Wq)[h * dk : (h + 1) * dk].T.astype(np.float64) / np.sqrt(dk)
            k = xb @ np.asarray(Wk)[h * dk : (h + 1) * dk].T.astype(np.float64)
            v = xb @ np.asarray(Wv)[h * dk : (h + 1) * dk].T.astype(np.float64)
            sc = q @ k.T
            sc = np.where(mrow[None, :] == 0, -1e9, sc)
            e = np.exp(sc - sc.max(-1, keepdims=True))
            p = e / e.sum(-1, keepdims=True)
            A[:, h * dk : (h + 1) * dk] = p @ v
        out[b] = A @ np.asarray(Wo).T.astype(np.float64)
    return (out + np.asarray(bo)[None, None, :]).astype(np.float32)


# ---------------------------------------------------------------------------
# harness entry point: full inputs in, full output out

_CACHED = {}


def kernel(x, mask, Wq, Wk, Wv, Wo, bo):
    """Multi-head attention on 8 TRN2 NeuronCores (head-parallel TP).

    Sharding: 2 heads per core (Wq/Wk/Wv split by head rows); keys are
    compacted host-side using the (host-visible) key mask so only kept
    keys flow through K/V projection, scores, exp and AV; scores/softmax
    /AV run in keys-on-partition layout with a valid-flag column in the
    V-augmented matmul providing the softmax denominator; 16 small
    AllToAll collectives redistribute the head-sharded attention output
    to seq-sharded form as each 512-query block completes; each core
    then computes its 1/8 of output rows against full Wo.
    """
    from concourse.bass_utils import run_bass_kernel_spmd

    x = np.ascontiguousarray(np.asarray(x, dtype=np.float32))
    mask = np.asarray(mask)
    Wq = np.asarray(Wq, dtype=np.float32)
    Wk = np.asarray(Wk, dtype=np.float32)
    Wv = np.asarray(Wv, dtype=np.float32)
    Wo = np.asarray(Wo, dtype=np.float32)
    bo = np.asarray(bo, dtype=np.float32)

    B, S, H = x.shape
    skp = compute_skp(mask, B, S)
    cfg = Cfg(B=B, S=S, H=H, SKP=skp)
    if _CACHED.get("skp") != skp:
        _CACHED["nc"] = build_nc(cfg)
        _CACHED["skp"] = skp
    nc = _CACHED["nc"]

    ins = make_inputs(cfg, x, mask, Wq, Wk, Wv, Wo)
    res = run_bass_kernel_spmd(nc, ins, list(range(cfg.ncores)))
    ys = [res.results[c]["y"] for c in range(cfg.ncores)]
    return assemble_output(cfg, ys, bo).astype(np.float32)


# revision 32
# speedup vs baseline: 1.2233x; 1.1517x over previous
"""MHA SPMD kernel v6 for TRN2 (8 cores, head-parallel, mask-compacted keys).

v6 over v5:
- host-side key compaction: only unmasked keys (padded to SKP, a multiple
  of 128) go through K/V projection, scores, exp and AV.  Padding keys
  have x=0 -> k=0 -> score 0 -> exp 1, and are excluded from both the
  numerator and the softmax denominator by a 0/1 valid flag in the V
  "ones" column.  mask~Bernoulli(0.5) makes this a ~2x cut in attention
  work.
- scores for the two heads are issued back-to-back as PE row-tiles
  (rows 0-63 / 64-127) so they execute concurrently; one [128,2,IB]
  PSUM tile holds both heads' scores for a key chunk.
- exp runs as one ACT instruction per key chunk covering both heads
  (1024 free elements) to amortize the ~352-cycle ACT startup.
- AV for both heads interleaved per key chunk; per-head M=65 (64 V dims
  + valid column) gives the softmax denominator for free.

Output row mapping (per core c):
  y[b*SPC + il*64 + r, :] = out[b, il*IB + c*64 + r, :].
"""

from dataclasses import dataclass

import numpy as np

import concourse.bass as bass
import concourse.bacc as bacc
import concourse.mybir as mybir
import concourse.tile as tile
from concourse.masks import make_identity

F16 = mybir.dt.float16
F32 = mybir.dt.float32
NP_F16 = np.float16


@dataclass
class Cfg:
    B: int = 4
    S: int = 2048
    H: int = 1024
    nh: int = 16
    ncores: int = 8
    IB: int = 512
    SKP: int = 1152  # padded kept-key count (multiple of 128)

    @property
    def dk(self):
        return self.H // self.nh

    @property
    def R(self):
        return self.B * self.S

    @property
    def SPC(self):
        return self.S // self.ncores

    @property
    def KC(self):
        return self.H // 128

    @property
    def JCK(self):
        return self.SKP // 128

    @property
    def NIL(self):
        return self.S // self.IB


def build_nc(cfg: Cfg, loop_n: int = 0, fake_a2a: bool = False,
             phases=('proj', 'attn', 'a2a', 'out'),
             attn_parts=('sc', 'exp', 'av', 'norm')) -> bass.Bass:
    assert cfg.dk == 64
    B, S, H, R, IB = cfg.B, cfg.S, cfg.H, cfg.R, cfg.IB
    KC, JCK, SPC, SKP = cfg.KC, cfg.JCK, cfg.SPC, cfg.SKP
    NC = cfg.ncores
    NIL = cfg.NIL
    assert S % IB == 0 and SPC % 128 == 0 and SKP % 128 == 0

    nc = bacc.Bacc("TRN2")

    xt = nc.declare_dram_parameter("xt", [H, R], F16, isOutput=False)
    xkt = nc.declare_dram_parameter("xkt", [H, B * SKP], F16, isOutput=False)
    wq = nc.declare_dram_parameter("wq_t", [H, 128], F16, isOutput=False)
    wk = nc.declare_dram_parameter("wk_t", [H, 128], F16, isOutput=False)
    wv = nc.declare_dram_parameter("wv_t", [H, 128], F16, isOutput=False)
    wo = nc.declare_dram_parameter("wo_t", [H, H], F16, isOutput=False)
    vld = nc.declare_dram_parameter("valid16", [128, B * JCK], F16,
                                    isOutput=False)
    y = nc.declare_dram_parameter("y", [B * SPC, H], F16, isOutput=True)

    CW = IB // NC                      # columns per A2A chunk shard (64)
    cc_in = [
        nc.dram_tensor(f"cc_in{b}", [NIL * NC * 128, CW], F16) for b in range(B)
    ]
    cc_out = [
        nc.dram_tensor(f"cc_out{b}", [NIL * NC * 128, CW], F16) for b in range(B)
    ]

    xt_r = xt[:].rearrange("(kc p) i -> p kc i", p=128)
    xkt_r = xkt[:].rearrange("(kc p) i -> p kc i", p=128)
    wq_r = wq[:].rearrange("(kc p) m -> p kc m", p=128)
    wk_r = wk[:].rearrange("(kc p) m -> p kc m", p=128)
    wv_r = wv[:].rearrange("(kc p) m -> p kc m", p=128)
    wo_r = wo[:].rearrange("(kc p) n -> p kc n", p=128)

    with tile.TileContext(nc) as tc:
        with tc.tile_pool(name="persist", bufs=1) as persist:
            wq_sb = persist.tile([128, KC, 128], F16)
            wk_sb = persist.tile([128, KC, 128], F16)
            wv_sb = persist.tile([128, KC, 128], F16)
            wo_sb = persist.tile([128, KC, H], F16)
            nc.scalar.dma_start(out=wq_sb[:], in_=wq_r)
            nc.scalar.dma_start(out=wk_sb[:], in_=wk_r)
            nc.scalar.dma_start(out=wv_sb[:], in_=wv_r)

            qt_sb = [persist.tile([128, S], F16, name=f"qt{b}") for b in range(B)]
            kt_sb = [persist.tile([128, SKP], F16, name=f"kt{b}") for b in range(B)]
            v_sb = [
                persist.tile([128, JCK, 130], F16, name=f"v{b}") for b in range(B)
            ]
            ones65 = persist.tile([65, 64], F16)
            nc.vector.memset(ones65[64:65, :], 1.0)
            ident = persist.tile([128, 128], F16)
            make_identity(nc, ident)
            # valid flags -> the two per-head "ones" columns of V (once;
            # the steady-state loop never rewrites these columns)
            for b in range(B):
                vsl = vld[:, bass.ds(b * JCK, JCK)].rearrange(
                    "p (n o) -> p n o", o=1
                )
                nc.gpsimd.dma_start(out=v_sb[b][:, :, 64:65], in_=vsl)
                nc.gpsimd.dma_start(out=v_sb[b][:, :, 129:130], in_=vsl)

            with (
                tc.tile_pool(name="xtp", bufs=3) as xtp,
                tc.tile_pool(name="xkp", bufs=1) as xkp,
                tc.tile_pool(name="ep", bufs=2) as ep,
                tc.tile_pool(name="rp", bufs=4) as rp,
                tc.tile_pool(name="asb", bufs=4) as asb,
                tc.tile_pool(name="agp", bufs=2) as agp,
                tc.tile_pool(name="ysb", bufs=2) as ysb,
                tc.tile_pool(name="pmm", bufs=1, space="PSUM") as pmm,
                tc.tile_pool(name="ps", bufs=2, space="PSUM") as ps,
                tc.tile_pool(name="po", bufs=2, space="PSUM") as po,
                tc.tile_pool(name="prb", bufs=1, space="PSUM") as prb,
            ):

                def q_units(b):
                    units = []
                    for ibl in range(S // IB):
                        isl = bass.ts(ibl, IB)
                        gsl = bass.ds(b * S + ibl * IB, IB)

                        def q_unit(b=b, isl=isl, gsl=gsl):
                            xt_t = xtp.tile(
                                [128, KC, IB], F16, tag="xt", name="xt_t"
                            )
                            nc.sync.dma_start(out=xt_t[:], in_=xt_r[:, :, gsl])
                            qp = pmm.tile([128, IB], F32, tag="mm", name="qp")
                            for kc in range(KC):
                                nc.tensor.matmul(
                                    qp[:], wq_sb[:, kc], xt_t[:, kc],
                                    start=(kc == 0), stop=(kc == KC - 1),
                                )
                            nc.vector.tensor_scalar_mul(
                                qt_sb[b][:, isl], qp[:], 0.125
                            )

                        units.append(q_unit)
                    return units

                def kv_units(b):
                    """K+V projection over the compacted (kept) keys.

                    One whole-batch xkt DMA, then per-512-block K and V
                    matmul units.  Batch 0's DMA goes on the (then idle)
                    scalar ring so it overlaps the q0 stream on sync.
                    """
                    units = []
                    xk_holder = {}

                    def kv_dma(b=b, xh=xk_holder):
                        xk_t = xkp.tile(
                            [128, KC, SKP], F16, tag="xk", name="xk_t"
                        )
                        gsl = bass.ds(b * SKP, SKP)
                        eng = nc.scalar if b == 0 else nc.sync
                        eng.dma_start(out=xk_t[:], in_=xkt_r[:, :, gsl])
                        xh["t"] = xk_t

                    units.append(kv_dma)
                    off = 0
                    while off < SKP:
                        n = min(IB, SKP - off)

                        def k_unit(b=b, off=off, n=n, xh=xk_holder):
                            kp = pmm.tile([128, IB], F32, tag="mm", name="kp")
                            for kc in range(KC):
                                nc.tensor.matmul(
                                    kp[:, 0:n], wk_sb[:, kc],
                                    xh["t"][:, kc, bass.ds(off, n)],
                                    start=(kc == 0), stop=(kc == KC - 1),
                                )
                            nc.vector.tensor_copy(
                                kt_sb[b][:, bass.ds(off, n)], kp[:, 0:n]
                            )

                        def v_unit(b=b, off=off, n=n, xh=xk_holder):
                            vtp = pmm.tile([128, IB], F32, tag="mm", name="vtp")
                            for kc in range(KC):
                                nc.tensor.matmul(
                                    vtp[:, 0:n], wv_sb[:, kc],
                                    xh["t"][:, kc, bass.ds(off, n)],
                                    start=(kc == 0), stop=(kc == KC - 1),
                                )
                            vt16 = xtp.tile(
                                [128, IB], F16, tag="vt16", name="vt16"
                            )
                            for t in range(n // 128):
                                nc.vector.tensor_copy(
                                    vt16[:, bass.ts(t, 128)],
                                    vtp[:, bass.ts(t, 128)],
                                )
                                vp = pmm.tile(
                                    [128, 128], F16, tag="mm", name="vp"
                                )
                                nc.tensor.transpose(
                                    vp[:], vt16[:, bass.ts(t, 128)], ident[:]
                                )
                                ch = off // 128 + t
                                nc.vector.tensor_copy(
                                    v_sb[b][:, ch, 0:64], vp[:, 0:64]
                                )
                                nc.vector.tensor_copy(
                                    v_sb[b][:, ch, 65:129], vp[:, 64:128]
                                )

                        units += [k_unit, v_unit]
                        off += n
                    return units

                def out_proj_units(b):
                    units = []
                    for it in range(SPC // 128):
                        holder = {}

                        def u0(b=b, it=it, hd=holder):
                            ag_t = agp.tile(
                                [128, KC, 128], F16, tag="ag", name="ag_t"
                            )
                            for half in range(128 // CW):
                                il = it * (128 // CW) + half
                                cc_r = cc_out[b][
                                    bass.ds(il * NC * 128, NC * 128), :
                                ].rearrange("(kc p) i -> p kc i", p=128)
                                nc.sync.dma_start(
                                    out=ag_t[:, :, bass.ts(half, CW)], in_=cc_r
                                )
                            y_t = ysb.tile([128, H], F16, tag="y", name="y_t")
                            hd["ag"], hd["y"] = ag_t, y_t
                            yp = pmm.tile([128, 512], F32, tag="mm", name="yp")
                            for kc in range(KC):
                                nc.tensor.matmul(
                                    yp[:], ag_t[:, kc], wo_sb[:, kc, 0:512],
                                    start=(kc == 0), stop=(kc == KC - 1),
                                )
                            nc.vector.tensor_copy(y_t[:, 0:512], yp[:])

                        def u1(b=b, it=it, hd=holder):
                            yp = pmm.tile([128, 512], F32, tag="mm", name="yp")
                            for kc in range(KC):
                                nc.tensor.matmul(
                                    yp[:], hd["ag"][:, kc],
                                    wo_sb[:, kc, 512:1024],
                                    start=(kc == 0), stop=(kc == KC - 1),
                                )
                            nc.vector.tensor_copy(hd["y"][:, 512:1024], yp[:])
                            nc.sync.dma_start(
                                out=y[bass.ds(b * SPC + it * 128, 128), :],
                                in_=hd["y"][:],
                            )

                        units += [u0, u1]
                    return units

                def attn_batch(b, fillers, pre_av=0):
                    fi = 0

                    def fill(n=1):
                        nonlocal fi
                        n = min(n, len(fillers) - fi)
                        for _ in range(n):
                            fillers[fi]()
                            fi += 1

                    def scores_chunk(il, jc, e_t):
                        qsl = bass.ts(il, IB)
                        sp = ps.tile([128, 2, IB], F32, tag="sp", name="sp")
                        for h in range(2):
                            hsl = bass.ds(h * 64, 64)
                            nc.tensor.matmul(
                                sp[:, h],
                                kt_sb[b][hsl, bass.ts(jc, 128)],
                                qt_sb[b][hsl, qsl],
                                start=True, stop=True,
                            )
                        if "exp" in attn_parts:
                            nc.scalar.activation(
                                e_t[:, jc], sp[:],
                                mybir.ActivationFunctionType.Exp,
                            )

                    # software pipeline: scores/exp for block il+1 are
                    # emitted interleaved with AV for block il, so ACT
                    # streams exps continuously while PE does AV work.
                    nblk = NIL + 1
                    per_blk = -(-len(fillers) // nblk) if fillers else 0
                    e_ts = {}
                    if "sc" in attn_parts:
                        e_ts[0] = ep.tile(
                            [128, JCK, 2, IB], F16, tag="e", name="e_t"
                        )
                        for jc in range(JCK):
                            scores_chunk(0, jc, e_ts[0])
                            if jc == JCK // 2:
                                fill(per_blk)
                    if fi < pre_av:
                        # data-dependency: units the first AV loop needs
                        fill(pre_av - fi)
                    for il in range(NIL):
                        e_t = e_ts.pop(il, None)
                        if "sc" in attn_parts and il + 1 < NIL:
                            e_ts[il + 1] = ep.tile(
                                [128, JCK, 2, IB], F16, tag="e", name="e_t"
                            )
                        if "av" in attn_parts:
                            o2 = [
                                po.tile([65, IB], F32, tag="oav", name=f"o2_{h}")
                                for h in range(2)
                            ]
                            for jc in range(JCK):
                                if "sc" in attn_parts and il + 1 < NIL:
                                    scores_chunk(il + 1, jc, e_ts[il + 1])
                                for h in range(2):
                                    nc.tensor.matmul(
                                        o2[h][:],
                                        v_sb[b][:, jc, bass.ds(h * 65, 65)],
                                        e_t[:, jc, h],
                                        start=(jc == 0), stop=(jc == JCK - 1),
                                    )
                                if jc == JCK // 2:
                                    fill(per_blk)
                        elif "sc" in attn_parts and il + 1 < NIL:
                            for jc in range(JCK):
                                scores_chunk(il + 1, jc, e_ts[il + 1])
                        if "norm" in attn_parts:
                            a_t = []
                            r16s = []
                            for h in range(2):
                                r32 = rp.tile([65, IB], F32, tag="r32", name="r32")
                                nc.vector.reciprocal(
                                    r32[64:65, :], o2[h][64:65, :]
                                )
                                r16 = rp.tile([65, IB], F16, tag="r16", name="r16")
                                nc.vector.tensor_copy(
                                    r16[64:65, :], r32[64:65, :]
                                )
                                r16s.append(r16)
                            # PE cover for the reciprocal chain latency so
                            # the rb matmuls don't stall the PE queue
                            fill(1)
                            at = asb.tile([64, 2, IB], F16, tag="a", name="a_t")
                            for h in range(2):
                                rb = prb.tile([64, IB], F32, tag="rb", name="rb")
                                nc.tensor.matmul(
                                    rb[:], ones65[64:65, :], r16s[h][64:65, :],
                                    start=True, stop=True,
                                )
                                rb_sb = rp.tile([64, IB], F32, tag="rbs", name="rb_sb")
                                nc.vector.tensor_copy(rb_sb[:], rb[:])
                                nc.vector.tensor_mul(
                                    at[:, h], o2[h][0:64, :], rb_sb[:]
                                )
                            a_t = at
                        if has("a2a"):
                            a2a_chunk(b, il, a_t)
                    fill(len(fillers))

                def a2a_chunk(b, il, a_t):
                    # cc_in[b][il*NC*128 + j*128 + h*64 + p, i]
                    #   = a_t[p, h, j*CW + i]
                    rows = bass.ds(il * NC * 128, NC * 128)
                    for h in range(2):
                        dst = cc_in[b][rows, :].rearrange(
                            "(j two p) i -> two p j i", j=NC, two=2
                        )[h]
                        src = a_t[:, h].rearrange("p (j i) -> p j i", j=NC)
                        nc.sync.dma_start(out=dst, in_=src)
                    if fake_a2a:
                        nc.gpsimd.dma_start(
                            out=cc_out[b][rows, :], in_=cc_in[b][rows, :]
                        )
                    else:
                        nc.gpsimd.collective_compute(
                            "AllToAll",
                            mybir.AluOpType.bypass,
                            replica_groups=[list(range(NC))],
                            ins=[cc_in[b][rows, :]],
                            outs=[cc_out[b][rows, :]],
                        )

                has = lambda p: p in phases

                def whole_kernel():
                    if not has("attn"):
                        if has("proj"):
                            for b in range(B):
                                for u in q_units(b) + kv_units(b):
                                    u()
                        if has("out"):
                            nc.scalar.dma_start(out=wo_sb[:], in_=wo_r)
                            for b in range(B):
                                for u in out_proj_units(b):
                                    u()
                        return
                    kv0 = kv_units(0) if has("proj") else []
                    if has("proj"):
                        # prologue: batch 0's Q, its kv DMA and K units
                        # (V units fill during attn 0, before its first AV)
                        for u in q_units(0) + kv0[0:1] + kv0[1::2]:
                            u()
                    nc.scalar.dma_start(out=wo_sb[:], in_=wo_r)
                    tail_units = []
                    for b in range(B):
                        fillers = []
                        pre_av = 0
                        if has("proj"):
                            if b == 0:
                                fillers += kv0[2::2]
                                pre_av = len(fillers)
                            if b + 1 < B:
                                fillers += q_units(b + 1) + kv_units(b + 1)
                        if has("out"):
                            # out-proj for batch pb fills during attn(pb+2)
                            # (collective for pb done during attn(pb+1))
                            if b >= 2:
                                fillers += out_proj_units(b - 2)
                            if b == B - 1:
                                fillers += out_proj_units(b - 1)
                                last = out_proj_units(b)
                                fillers += last[:2]
                                tail_units = last[2:]
                        attn_batch(b, fillers, pre_av=pre_av)
                    for u in tail_units:
                        u()

                def attn_prereq():
                    for b in range(B):
                        for u in q_units(b) + kv_units(b):
                            u()

                if loop_n > 0:
                    if has("attn") and not has("proj"):
                        attn_prereq()
                    with tc.For_i(0, loop_n):
                        whole_kernel()
                else:
                    whole_kernel()

    nc.finalize()
    return nc


# ---------------------------------------------------------------------------


def compute_skp(mask, B, S):
    m = (np.asarray(mask).reshape(B, S) != 0)
    kept = int(m.sum(1).max())
    skp = max(128, -(-kept // 128) * 128)
    return min(skp, S)


def make_inputs(cfg: Cfg, x, mask, Wq, Wk, Wv, Wo):
    B, S, H, NC, SKP, JCK = cfg.B, cfg.S, cfg.H, cfg.ncores, cfg.SKP, cfg.JCK
    x = np.asarray(x, dtype=np.float32)
    xt = np.ascontiguousarray(x.reshape(B * S, H).T.astype(NP_F16))
    wo_t = np.ascontiguousarray(np.asarray(Wo).T.astype(NP_F16))

    m = (np.asarray(mask).reshape(B, S) != 0)
    xk = np.zeros((B, SKP, H), np.float32)
    valid = np.zeros((B, JCK * 128), NP_F16)
    for b in range(B):
        idx = np.nonzero(m[b])[0]
        assert len(idx) <= SKP, (len(idx), SKP)
        xk[b, : len(idx)] = x[b, idx]
        valid[b, : len(idx)] = 1.0
    xkt = np.ascontiguousarray(xk.reshape(B * SKP, H).T.astype(NP_F16))
    # valid16[p, b*JCK + c] = valid[b, c*128 + p]
    v16 = np.ascontiguousarray(
        valid.reshape(B * JCK, 128).T.astype(NP_F16)
    )

    ins = []
    for c in range(NC):
        blk = slice(c * 128, (c + 1) * 128)
        ins.append(
            {
                "xt": xt,
                "xkt": xkt,
                "wq_t": np.ascontiguousarray(np.asarray(Wq)[blk, :].T.astype(NP_F16)),
                "wk_t": np.ascontiguousarray(np.asarray(Wk)[blk, :].T.astype(NP_F16)),
                "wv_t": np.ascontiguousarray(np.asarray(Wv)[blk, :].T.astype(NP_F16)),
                "wo_t": wo_t,
                "valid16": v16,
            }
        )
    return ins


def assemble_output(cfg: Cfg, per_core_y, bo):
    B, S, H, SPC, IB = cfg.B, cfg.S, cfg.H, cfg.SPC, cfg.IB
    NC = cfg.ncores
    CW = IB // NC
    NIL = S // IB
    out = np.empty((B, S, H), np.float32)
    for c, yc in enumerate(per_core_y):
        yc = np.asarray(yc).reshape(B, NIL, CW, H)
        for b in range(B):
            for il in range(NIL):
                out[b, il * IB + c * CW : il * IB + (c + 1) * CW] = yc[b, il]
    out += np.asarray(bo, dtype=np.float32)[None, None, :]
    return out


def reference_np(cfg: Cfg, x, mask, Wq, Wk, Wv, Wo, bo):
    B, S, H, nh, dk = cfg.B, cfg.S, cfg.H, cfg.nh, cfg.dk
    xf = np.asarray(x).reshape(B * S, H).astype(np.float64)
    out = np.zeros((B, S, H), np.float64)
    for b in range(B):
        xb = xf[b * S : (b + 1) * S]
        mrow = np.asarray(mask).reshape(B, S)[b]
        A = np.zeros((S, H), np.float64)
        for h in range(nh):
            q = xb @ np.asarray(Wq)[h * dk : (h + 1) * dk].T.astype(np.float64) / np.sqrt(dk)
            k = xb @ np.asarray(Wk)[h * dk : (h + 1) * dk].T.astype(np.float64)
            v = xb @ np.asarray(Wv)[h * dk : (h + 1) * dk].T.astype(np.float64)
            sc = q @ k.T
            sc = np.where(mrow[None, :] == 0, -1e9, sc)
            e = np.exp(sc - sc.max(-1, keepdims=True))
            p = e / e.sum(-1, keepdims=True)
            A[:, h * dk : (h + 1) * dk] = p @ v
        out[b] = A @ np.asarray(Wo).T.astype(np.float64)
    return (out + np.asarray(bo)[None, None, :]).astype(np.float32)


# ---------------------------------------------------------------------------
# harness entry point: full inputs in, full output out

_CACHED = {}


def kernel(x, mask, Wq, Wk, Wv, Wo, bo):
    """Multi-head attention on 8 TRN2 NeuronCores (head-parallel TP).

    Sharding: 2 heads per core (Wq/Wk/Wv split by head rows); keys are
    compacted host-side using the (host-visible) key mask so only kept
    keys flow through K/V projection, scores, exp and AV; scores/softmax
    /AV run in keys-on-partition layout with a valid-flag column in the
    V-augmented matmul providing the softmax denominator; 16 small
    AllToAll collectives redistribute the head-sharded attention output
    to seq-sharded form as each 512-query block completes; each core
    then computes its 1/8 of output rows against full Wo.
    """
    from concourse.bass_utils import run_bass_kernel_spmd

    x = np.ascontiguousarray(np.asarray(x, dtype=np.float32))
    mask = np.asarray(mask)
    Wq = np.asarray(Wq, dtype=np.float32)
    Wk = np.asarray(Wk, dtype=np.float32)
    Wv = np.asarray(Wv, dtype=np.float32)
    Wo = np.asarray(Wo, dtype=np.float32)
    bo = np.asarray(bo, dtype=np.float32)

    B, S, H = x.shape
    skp = compute_skp(mask, B, S)
    cfg = Cfg(B=B, S=S, H=H, SKP=skp)
    if _CACHED.get("skp") != skp:
        _CACHED["nc"] = build_nc(cfg)
        _CACHED["skp"] = skp
    nc = _CACHED["nc"]

    ins = make_inputs(cfg, x, mask, Wq, Wk, Wv, Wo)
    res = run_bass_kernel_spmd(nc, ins, list(range(cfg.ncores)))
    ys = [res.results[c]["y"] for c in range(cfg.ncores)]
    return assemble_output(cfg, ys, bo).astype(np.float32)


# revision 33
# speedup vs baseline: 1.3154x; 1.0753x over previous
"""MHA SPMD kernel v6 for TRN2 (8 cores, head-parallel, mask-compacted keys).

v6 over v5:
- host-side key compaction: only unmasked keys (padded to SKP, a multiple
  of 128) go through K/V projection, scores, exp and AV.  Padding keys
  have x=0 -> k=0 -> score 0 -> exp 1, and are excluded from both the
  numerator and the softmax denominator by a 0/1 valid flag in the V
  "ones" column.  mask~Bernoulli(0.5) makes this a ~2x cut in attention
  work.
- scores for the two heads are issued back-to-back as PE row-tiles
  (rows 0-63 / 64-127) so they execute concurrently; one [128,2,IB]
  PSUM tile holds both heads' scores for a key chunk.
- exp runs as one ACT instruction per key chunk covering both heads
  (1024 free elements) to amortize the ~352-cycle ACT startup.
- AV for both heads interleaved per key chunk; per-head M=65 (64 V dims
  + valid column) gives the softmax denominator for free.

Output row mapping (per core c):
  y[b*SPC + il*64 + r, :] = out[b, il*IB + c*64 + r, :].
"""

from dataclasses import dataclass

import numpy as np

import concourse.bass as bass
import concourse.bacc as bacc
import concourse.mybir as mybir
import concourse.tile as tile
from concourse.masks import make_identity

F16 = mybir.dt.float16
F32 = mybir.dt.float32
NP_F16 = np.float16


@dataclass
class Cfg:
    B: int = 4
    S: int = 2048
    H: int = 1024
    nh: int = 16
    ncores: int = 8
    IB: int = 512
    SKP: int = 1152  # padded kept-key count (multiple of 128)

    @property
    def dk(self):
        return self.H // self.nh

    @property
    def R(self):
        return self.B * self.S

    @property
    def SPC(self):
        return self.S // self.ncores

    @property
    def KC(self):
        return self.H // 128

    @property
    def JCK(self):
        return self.SKP // 128

    @property
    def NIL(self):
        return self.S // self.IB


def build_nc(cfg: Cfg, loop_n: int = 0, fake_a2a: bool = False,
             phases=('proj', 'attn', 'a2a', 'out'),
             attn_parts=('sc', 'exp', 'av', 'norm')) -> bass.Bass:
    assert cfg.dk == 64
    B, S, H, R, IB = cfg.B, cfg.S, cfg.H, cfg.R, cfg.IB
    KC, JCK, SPC, SKP = cfg.KC, cfg.JCK, cfg.SPC, cfg.SKP
    NC = cfg.ncores
    NIL = cfg.NIL
    assert S % IB == 0 and SPC % 128 == 0 and SKP % 128 == 0

    nc = bacc.Bacc("TRN2")

    xt = nc.declare_dram_parameter("xt", [H, R], F16, isOutput=False)
    xkt = nc.declare_dram_parameter("xkt", [H, B * SKP], F16, isOutput=False)
    wq = nc.declare_dram_parameter("wq_t", [H, 128], F16, isOutput=False)
    wk = nc.declare_dram_parameter("wk_t", [H, 128], F16, isOutput=False)
    wv = nc.declare_dram_parameter("wv_t", [H, 128], F16, isOutput=False)
    wo = nc.declare_dram_parameter("wo_t", [H, H], F16, isOutput=False)
    vld = nc.declare_dram_parameter("valid16", [128, B * JCK], F16,
                                    isOutput=False)
    y = nc.declare_dram_parameter("y", [B * SPC, H], F16, isOutput=True)

    CW = IB // NC                      # columns per A2A chunk shard (64)
    cc_in = [
        nc.dram_tensor(f"cc_in{b}", [NIL * NC * 128, CW], F16) for b in range(B)
    ]
    cc_out = [
        nc.dram_tensor(f"cc_out{b}", [NIL * NC * 128, CW], F16) for b in range(B)
    ]

    xt_r = xt[:].rearrange("(kc p) i -> p kc i", p=128)
    xkt_r = xkt[:].rearrange("(kc p) i -> p kc i", p=128)
    wq_r = wq[:].rearrange("(kc p) m -> p kc m", p=128)
    wk_r = wk[:].rearrange("(kc p) m -> p kc m", p=128)
    wv_r = wv[:].rearrange("(kc p) m -> p kc m", p=128)
    wo_r = wo[:].rearrange("(kc p) n -> p kc n", p=128)

    with tile.TileContext(nc) as tc:
        with tc.tile_pool(name="persist", bufs=1) as persist:
            wq_sb = persist.tile([128, KC, 128], F16)
            wk_sb = persist.tile([128, KC, 128], F16)
            wv_sb = persist.tile([128, KC, 128], F16)
            wo_sb = persist.tile([128, KC, H], F16)
            nc.scalar.dma_start(out=wq_sb[:], in_=wq_r)
            nc.scalar.dma_start(out=wk_sb[:], in_=wk_r)
            nc.scalar.dma_start(out=wv_sb[:], in_=wv_r)

            qt_sb = [persist.tile([128, S], F16, name=f"qt{b}") for b in range(B)]
            kt_sb = [persist.tile([128, SKP], F16, name=f"kt{b}") for b in range(B)]
            v_sb = [
                persist.tile([128, JCK, 130], F16, name=f"v{b}") for b in range(B)
            ]
            ones65 = persist.tile([65, 64], F16)
            nc.vector.memset(ones65[64:65, :], 1.0)
            ident = persist.tile([128, 128], F16)
            make_identity(nc, ident)
            # valid flags -> the two per-head "ones" columns of V (once;
            # the steady-state loop never rewrites these columns)
            for b in range(B):
                vsl = vld[:, bass.ds(b * JCK, JCK)].rearrange(
                    "p (n o) -> p n o", o=1
                )
                nc.gpsimd.dma_start(out=v_sb[b][:, :, 64:65], in_=vsl)
                nc.gpsimd.dma_start(out=v_sb[b][:, :, 129:130], in_=vsl)

            with (
                tc.tile_pool(name="xtp", bufs=3) as xtp,
                tc.tile_pool(name="xkp", bufs=1) as xkp,
                tc.tile_pool(name="ep", bufs=2) as ep,
                tc.tile_pool(name="rp", bufs=4) as rp,
                tc.tile_pool(name="asb", bufs=4) as asb,
                tc.tile_pool(name="agp", bufs=2) as agp,
                tc.tile_pool(name="ysb", bufs=2) as ysb,
                tc.tile_pool(name="pmm", bufs=1, space="PSUM") as pmm,
                tc.tile_pool(name="ps", bufs=2, space="PSUM") as ps,
                tc.tile_pool(name="po", bufs=2, space="PSUM") as po,
                tc.tile_pool(name="prb", bufs=1, space="PSUM") as prb,
            ):

                def q_units(b):
                    units = []
                    for ibl in range(S // IB):
                        isl = bass.ts(ibl, IB)
                        gsl = bass.ds(b * S + ibl * IB, IB)

                        def q_unit(b=b, isl=isl, gsl=gsl):
                            xt_t = xtp.tile(
                                [128, KC, IB], F16, tag="xt", name="xt_t"
                            )
                            nc.sync.dma_start(out=xt_t[:], in_=xt_r[:, :, gsl])
                            qp = pmm.tile([128, IB], F32, tag="mm", name="qp")
                            for kc in range(KC):
                                nc.tensor.matmul(
                                    qp[:], wq_sb[:, kc], xt_t[:, kc],
                                    start=(kc == 0), stop=(kc == KC - 1),
                                )
                            nc.vector.tensor_scalar_mul(
                                qt_sb[b][:, isl], qp[:], 0.125
                            )

                        units.append(q_unit)
                    return units

                def kv_units(b):
                    """K+V projection over the compacted (kept) keys.

                    One whole-batch xkt DMA, then per-512-block K and V
                    matmul units.  Batch 0's DMA goes on the (then idle)
                    scalar ring so it overlaps the q0 stream on sync.
                    """
                    units = []
                    xk_holder = {}

                    def kv_dma(b=b, xh=xk_holder):
                        xk_t = xkp.tile(
                            [128, KC, SKP], F16, tag="xk", name="xk_t"
                        )
                        gsl = bass.ds(b * SKP, SKP)
                        eng = nc.scalar if b == 0 else nc.sync
                        eng.dma_start(out=xk_t[:], in_=xkt_r[:, :, gsl])
                        xh["t"] = xk_t

                    units.append(kv_dma)
                    off = 0
                    while off < SKP:
                        n = min(IB, SKP - off)

                        def k_unit(b=b, off=off, n=n, xh=xk_holder):
                            kp = pmm.tile([128, IB], F32, tag="mm", name="kp")
                            for kc in range(KC):
                                nc.tensor.matmul(
                                    kp[:, 0:n], wk_sb[:, kc],
                                    xh["t"][:, kc, bass.ds(off, n)],
                                    start=(kc == 0), stop=(kc == KC - 1),
                                )
                            nc.vector.tensor_copy(
                                kt_sb[b][:, bass.ds(off, n)], kp[:, 0:n]
                            )

                        def v_unit(b=b, off=off, n=n, xh=xk_holder):
                            vtp = pmm.tile([128, IB], F32, tag="mm", name="vtp")
                            for kc in range(KC):
                                nc.tensor.matmul(
                                    vtp[:, 0:n], wv_sb[:, kc],
                                    xh["t"][:, kc, bass.ds(off, n)],
                                    start=(kc == 0), stop=(kc == KC - 1),
                                )
                            vt16 = xtp.tile(
                                [128, IB], F16, tag="vt16", name="vt16"
                            )
                            for t in range(n // 128):
                                nc.vector.tensor_copy(
                                    vt16[:, bass.ts(t, 128)],
                                    vtp[:, bass.ts(t, 128)],
                                )
                                vp = pmm.tile(
                                    [128, 128], F16, tag="mm", name="vp"
                                )
                                nc.tensor.transpose(
                                    vp[:], vt16[:, bass.ts(t, 128)], ident[:]
                                )
                                ch = off // 128 + t
                                nc.vector.tensor_copy(
                                    v_sb[b][:, ch, 0:64], vp[:, 0:64]
                                )
                                nc.vector.tensor_copy(
                                    v_sb[b][:, ch, 65:129], vp[:, 64:128]
                                )

                        units += [k_unit, v_unit]
                        off += n
                    return units

                def out_proj_units(b):
                    units = []
                    for it in range(SPC // 128):
                        holder = {}

                        def u0(b=b, it=it, hd=holder):
                            ag_t = agp.tile(
                                [128, KC, 128], F16, tag="ag", name="ag_t"
                            )
                            for half in range(128 // CW):
                                il = it * (128 // CW) + half
                                cc_r = cc_out[b][
                                    bass.ds(il * NC * 128, NC * 128), :
                                ].rearrange("(kc p) i -> p kc i", p=128)
                                nc.sync.dma_start(
                                    out=ag_t[:, :, bass.ts(half, CW)], in_=cc_r
                                )
                            y_t = ysb.tile([128, H], F16, tag="y", name="y_t")
                            hd["ag"], hd["y"] = ag_t, y_t
                            yp = pmm.tile([128, 512], F32, tag="mm", name="yp")
                            for kc in range(KC):
                                nc.tensor.matmul(
                                    yp[:], ag_t[:, kc], wo_sb[:, kc, 0:512],
                                    start=(kc == 0), stop=(kc == KC - 1),
                                )
                            nc.vector.tensor_copy(y_t[:, 0:512], yp[:])

                        def u1(b=b, it=it, hd=holder):
                            yp = pmm.tile([128, 512], F32, tag="mm", name="yp")
                            for kc in range(KC):
                                nc.tensor.matmul(
                                    yp[:], hd["ag"][:, kc],
                                    wo_sb[:, kc, 512:1024],
                                    start=(kc == 0), stop=(kc == KC - 1),
                                )
                            nc.vector.tensor_copy(hd["y"][:, 512:1024], yp[:])
                            nc.sync.dma_start(
                                out=y[bass.ds(b * SPC + it * 128, 128), :],
                                in_=hd["y"][:],
                            )

                        units += [u0, u1]
                    return units

                def attn_batch(b, fillers, pre_av=0):
                    fi = 0

                    def fill(n=1):
                        nonlocal fi
                        n = min(n, len(fillers) - fi)
                        for _ in range(n):
                            fillers[fi]()
                            fi += 1

                    def scores_chunk(il, jc, e_t):
                        qsl = bass.ts(il, IB)
                        sp = ps.tile([128, 2, IB], F32, tag="sp", name="sp")
                        for h in range(2):
                            hsl = bass.ds(h * 64, 64)
                            nc.tensor.matmul(
                                sp[:, h],
                                kt_sb[b][hsl, bass.ts(jc, 128)],
                                qt_sb[b][hsl, qsl],
                                start=True, stop=True,
                            )
                        if "exp" in attn_parts:
                            nc.scalar.activation(
                                e_t[:, jc], sp[:],
                                mybir.ActivationFunctionType.Exp,
                            )

                    # software pipeline: scores/exp for block il+1 are
                    # emitted interleaved with AV for block il, so ACT
                    # streams exps continuously while PE does AV work.
                    nblk = NIL + 1
                    per_blk = -(-len(fillers) // nblk) if fillers else 0
                    e_ts = {}
                    if "sc" in attn_parts:
                        e_ts[0] = ep.tile(
                            [128, JCK, 2, IB], F16, tag="e", name="e_t"
                        )
                        for jc in range(JCK):
                            scores_chunk(0, jc, e_ts[0])
                            if jc == JCK // 2:
                                fill(per_blk)
                    if fi < pre_av:
                        # data-dependency: units the first AV loop needs
                        fill(pre_av - fi)
                    for il in range(NIL):
                        e_t = e_ts.pop(il, None)
                        if "sc" in attn_parts and il + 1 < NIL:
                            e_ts[il + 1] = ep.tile(
                                [128, JCK, 2, IB], F16, tag="e", name="e_t"
                            )
                        if "av" in attn_parts:
                            o2 = [
                                po.tile([65, IB], F32, tag="oav", name=f"o2_{h}")
                                for h in range(2)
                            ]
                            for jc in range(JCK):
                                if "sc" in attn_parts and il + 1 < NIL:
                                    scores_chunk(il + 1, jc, e_ts[il + 1])
                                for h in range(2):
                                    nc.tensor.matmul(
                                        o2[h][:],
                                        v_sb[b][:, jc, bass.ds(h * 65, 65)],
                                        e_t[:, jc, h],
                                        start=(jc == 0), stop=(jc == JCK - 1),
                                    )
                                if jc == JCK // 2:
                                    fill(per_blk)
                        elif "sc" in attn_parts and il + 1 < NIL:
                            for jc in range(JCK):
                                scores_chunk(il + 1, jc, e_ts[il + 1])
                        if "norm" in attn_parts:
                            a_t = []
                            r16s = []
                            for h in range(2):
                                r32 = rp.tile([65, IB], F32, tag="r32", name="r32")
                                nc.vector.reciprocal(
                                    r32[64:65, :], o2[h][64:65, :]
                                )
                                r16 = rp.tile([65, IB], F16, tag="r16", name="r16")
                                nc.vector.tensor_copy(
                                    r16[64:65, :], r32[64:65, :]
                                )
                                r16s.append(r16)
                            # PE cover for the reciprocal chain latency so
                            # the rb matmuls don't stall the PE queue
                            fill(1)
                            at = asb.tile([64, 2, IB], F16, tag="a", name="a_t")
                            for h in range(2):
                                rb = prb.tile([64, IB], F32, tag="rb", name="rb")
                                nc.tensor.matmul(
                                    rb[:], ones65[64:65, :], r16s[h][64:65, :],
                                    start=True, stop=True,
                                )
                                rb_sb = rp.tile([64, IB], F32, tag="rbs", name="rb_sb")
                                nc.vector.tensor_copy(rb_sb[:], rb[:])
                                nc.vector.tensor_mul(
                                    at[:, h], o2[h][0:64, :], rb_sb[:]
                                )
                            a_t = at
                        if has("a2a"):
                            a2a_chunk(b, il, a_t)
                    fill(len(fillers))

                def a2a_chunk(b, il, a_t):
                    # cc_in[b][il*NC*128 + j*128 + h*64 + p, i]
                    #   = a_t[p, h, j*CW + i]
                    rows = bass.ds(il * NC * 128, NC * 128)
                    for h in range(2):
                        dst = cc_in[b][rows, :].rearrange(
                            "(j two p) i -> two p j i", j=NC, two=2
                        )[h]
                        src = a_t[:, h].rearrange("p (j i) -> p j i", j=NC)
                        nc.sync.dma_start(out=dst, in_=src)
                    if fake_a2a:
                        nc.sync.dma_start(
                            out=cc_out[b][rows, :], in_=cc_in[b][rows, :]
                        )
                    else:
                        nc.gpsimd.collective_compute(
                            "AllToAll",
                            mybir.AluOpType.bypass,
                            replica_groups=[list(range(NC))],
                            ins=[cc_in[b][rows, :]],
                            outs=[cc_out[b][rows, :]],
                        )

                has = lambda p: p in phases

                def whole_kernel():
                    if not has("attn"):
                        if has("proj"):
                            for b in range(B):
                                for u in q_units(b) + kv_units(b):
                                    u()
                        if has("out"):
                            nc.scalar.dma_start(out=wo_sb[:], in_=wo_r)
                            for b in range(B):
                                for u in out_proj_units(b):
                                    u()
                        return
                    kv0 = kv_units(0) if has("proj") else []
                    if has("proj"):
                        # prologue: batch 0's Q, its kv DMA and K units
                        # (V units fill during attn 0, before its first AV)
                        for u in q_units(0) + kv0[0:1] + kv0[1::2]:
                            u()
                    nc.scalar.dma_start(out=wo_sb[:], in_=wo_r)
                    tail_units = []
                    for b in range(B):
                        fillers = []
                        pre_av = 0
                        if has("proj"):
                            if b == 0:
                                fillers += kv0[2::2]
                                pre_av = len(fillers)
                            if b + 1 < B:
                                fillers += q_units(b + 1) + kv_units(b + 1)
                        if has("out"):
                            # out-proj for batch pb fills during attn(pb+2)
                            # (collective for pb done during attn(pb+1))
                            if b >= 2:
                                fillers += out_proj_units(b - 2)
                            if b == B - 1:
                                fillers += out_proj_units(b - 1)
                                last = out_proj_units(b)
                                fillers += last[:2]
                                tail_units = last[2:]
                        attn_batch(b, fillers, pre_av=pre_av)
                    for u in tail_units:
                        u()

                def attn_prereq():
                    for b in range(B):
                        for u in q_units(b) + kv_units(b):
                            u()

                if loop_n > 0:
                    if has("attn") and not has("proj"):
                        attn_prereq()
                    with tc.For_i(0, loop_n):
                        whole_kernel()
                else:
                    whole_kernel()

    nc.finalize()
    return nc


# ---------------------------------------------------------------------------


def compute_skp(mask, B, S):
    m = (np.asarray(mask).reshape(B, S) != 0)
    kept = int(m.sum(1).max())
    skp = max(128, -(-kept // 128) * 128)
    return min(skp, S)


def make_inputs(cfg: Cfg, x, mask, Wq, Wk, Wv, Wo):
    B, S, H, NC, SKP, JCK = cfg.B, cfg.S, cfg.H, cfg.ncores, cfg.SKP, cfg.JCK
    x = np.asarray(x, dtype=np.float32)
    xt = np.ascontiguousarray(x.reshape(B * S, H).T.astype(NP_F16))
    wo_t = np.ascontiguousarray(np.asarray(Wo).T.astype(NP_F16))

    m = (np.asarray(mask).reshape(B, S) != 0)
    xk = np.zeros((B, SKP, H), np.float32)
    valid = np.zeros((B, JCK * 128), NP_F16)
    for b in range(B):
        idx = np.nonzero(m[b])[0]
        assert len(idx) <= SKP, (len(idx), SKP)
        xk[b, : len(idx)] = x[b, idx]
        valid[b, : len(idx)] = 1.0
    xkt = np.ascontiguousarray(xk.reshape(B * SKP, H).T.astype(NP_F16))
    # valid16[p, b*JCK + c] = valid[b, c*128 + p]
    v16 = np.ascontiguousarray(
        valid.reshape(B * JCK, 128).T.astype(NP_F16)
    )

    ins = []
    for c in range(NC):
        blk = slice(c * 128, (c + 1) * 128)
        ins.append(
            {
                "xt": xt,
                "xkt": xkt,
                "wq_t": np.ascontiguousarray(np.asarray(Wq)[blk, :].T.astype(NP_F16)),
                "wk_t": np.ascontiguousarray(np.asarray(Wk)[blk, :].T.astype(NP_F16)),
                "wv_t": np.ascontiguousarray(np.asarray(Wv)[blk, :].T.astype(NP_F16)),
                "wo_t": wo_t,
                "valid16": v16,
            }
        )
    return ins


def assemble_output(cfg: Cfg, per_core_y, bo):
    B, S, H, SPC, IB = cfg.B, cfg.S, cfg.H, cfg.SPC, cfg.IB
    NC = cfg.ncores
    CW = IB // NC
    NIL = S // IB
    out = np.empty((B, S, H), np.float32)
    for c, yc in enumerate(per_core_y):
        yc = np.asarray(yc).reshape(B, NIL, CW, H)
        for b in range(B):
            for il in range(NIL):
                out[b, il * IB + c * CW : il * IB + (c + 1) * CW] = yc[b, il]
    out += np.asarray(bo, dtype=np.float32)[None, None, :]
    return out


def reference_np(cfg: Cfg, x, mask, Wq, Wk, Wv, Wo, bo):
    B, S, H, nh, dk = cfg.B, cfg.S, cfg.H, cfg.nh, cfg.dk
    xf = np.asarray(x).reshape(B * S, H).astype(np.float64)
    out = np.zeros((B, S, H), np.float64)
    for b in range(B):
        xb = xf[b * S : (b + 1) * S]
        mrow = np.asarray(mask).reshape(B, S)[b]
        A = np.zeros((S, H), np.float64)
        for h in range(nh):
            q = xb @ np.asarray(Wq)[h * dk : (h + 1) * dk].T.astype(np.float64) / np.sqrt(dk)
            k = xb @ np.asarray(Wk)[h * dk : (h + 1) * dk].T.astype(np.float64)
            v = xb @ np.asarray(Wv)[h * dk : (h + 1) * dk].T.astype(np.float64)
            sc = q @ k.T
            sc = np.where(mrow[None, :] == 0, -1e9, sc)
            e = np.exp(sc - sc.max(-1, keepdims=True))
            p = e / e.sum(-1, keepdims=True)
            A[:, h * dk : (h + 1) * dk] = p @ v
        out[b] = A @ np.asarray(Wo).T.astype(np.float64)
    return (out + np.asarray(bo)[None, None, :]).astype(np.float32)


# ---------------------------------------------------------------------------
# harness entry point: full inputs in, full output out

_CACHED = {}


def kernel(x, mask, Wq, Wk, Wv, Wo, bo):
    """Multi-head attention on 8 TRN2 NeuronCores (head-parallel TP).

    Sharding: 2 heads per core (Wq/Wk/Wv split by head rows); keys are
    compacted host-side using the (host-visible) key mask so only kept
    keys flow through K/V projection, scores, exp and AV; scores/softmax
    /AV run in keys-on-partition layout with a valid-flag column in the
    V-augmented matmul providing the softmax denominator; 16 small
    AllToAll collectives redistribute the head-sharded attention output
    to seq-sharded form as each 512-query block completes; each core
    then computes its 1/8 of output rows against full Wo.
    """
    from concourse.bass_utils import run_bass_kernel_spmd

    x = np.ascontiguousarray(np.asarray(x, dtype=np.float32))
    mask = np.asarray(mask)
    Wq = np.asarray(Wq, dtype=np.float32)
    Wk = np.asarray(Wk, dtype=np.float32)
    Wv = np.asarray(Wv, dtype=np.float32)
    Wo = np.asarray(Wo, dtype=np.float32)
    bo = np.asarray(bo, dtype=np.float32)

    B, S, H = x.shape
    skp = compute_skp(mask, B, S)
    cfg = Cfg(B=B, S=S, H=H, SKP=skp)
    if _CACHED.get("skp") != skp:
        _CACHED["nc"] = build_nc(cfg)
        _CACHED["skp"] = skp
    nc = _CACHED["nc"]

    ins = make_inputs(cfg, x, mask, Wq, Wk, Wv, Wo)
    res = run_bass_kernel_spmd(nc, ins, list(range(cfg.ncores)))
    ys = [res.results[c]["y"] for c in range(cfg.ncores)]
    return assemble_output(cfg, ys, bo).astype(np.float32)
